# revision 5
# baseline (speedup 1.0000x reference)
"""MoE routing kernel (nn_Dense_69045894250875) for 8 Trainium2 NeuronCores.

reference:  y = tanh(einsum('bloi,bli->blo', weight[channels], x) + bias[channels]) + x
            returns (y, channels)

Strategy (data-parallel over batch, 4 batches = 4096 tokens per core):
  1. On-device counting sort of tokens by channel (rank via strict-triangular
     matmuls + histogram prefix), producing dest[t] = c_t*128 + global_rank.
  2. Indirect-DMA scatter of x rows into a capacity-padded sorted DRAM table
     (128 slots per expert).
  3. Per-expert 64x64 GEMM on PE (fp32, exact): y_e = x_e @ W_e^T + b_e,
     tanh on ScalarE, residual add on VectorE.
  4. Dense store of sorted y, indirect-DMA gather back into token order.

All compute is on-device; the host only reshapes/shards inputs and
reassembles the output.
"""
import sys

sys.path.insert(0, "/opt/trn_rl_repo")

import numpy as np

import concourse.bass as bass
import concourse.mybir as mybir
import concourse.tile as tile
from concourse import bacc
from concourse.bass_utils import run_bass_kernel_spmd

P = 128          # partitions / slots per expert (capacity)
NB = 32          # token blocks per core
T = P * NB       # tokens per core = 4096
C = 64           # experts
D = 64           # in/out features
S = C * P        # sorted table slots = 8192
NCORES = 8

f32 = mybir.dt.float32
i32 = mybir.dt.int32

TRACE = False          # test harness sets True (requires NTFF hook installed)
LAST_RESULTS = None    # test harness reads this

_COMPILED = None


def _strict_upper(nc, ap, n):
    """ap[k, m] = 1.0 if k < m else 0.0  (k = partition, m = free)."""
    nc.gpsimd.memset(ap, 0.0)
    nc.gpsimd.affine_select(
        out=ap, in_=ap,
        compare_op=mybir.AluOpType.is_ge,   # keep 0 where k-m >= 0, else fill 1
        fill=1.0, base=0,
        pattern=[[-1, n]], channel_multiplier=1,
    )


def _build():
    nc = bacc.Bacc("TRN2", target_bir_lowering=False, debug=False,
                   num_devices=NCORES)

    x_d = nc.dram_tensor("x", [T, D], f32, kind="ExternalInput")
    ch_d = nc.dram_tensor("ch", [P, NB], i32, kind="ExternalInput")
    w_d = nc.dram_tensor("w", [C * D, D], f32, kind="ExternalInput")
    b_d = nc.dram_tensor("b", [C, D], f32, kind="ExternalInput")
    y_d = nc.dram_tensor("y", [T, D], f32, kind="ExternalOutput")

    xs_d = nc.dram_tensor("xs_scratch", [S, D], f32)   # sorted x (padded)
    ys_d = nc.dram_tensor("ys_scratch", [S, D], f32)   # sorted y (padded)

    with tile.TileContext(nc) as tc:
        with tc.tile_pool(name="persist", bufs=1) as pp, \
             tc.tile_pool(name="wload", bufs=3) as wl, \
             tc.tile_pool(name="tmp", bufs=3) as tp:

            # ---------------- constants ----------------
            ident = pp.tile([P, P], f32, tag="ident")
            from concourse.masks import make_identity
            make_identity(nc, ident[:])

            u128 = pp.tile([P, P], f32, tag="u128")
            _strict_upper(nc, u128[:], P)
            u32 = pp.tile([NB, NB], f32, tag="u32")
            _strict_upper(nc, u32[:], NB)

            iota_i = pp.tile([P, C], i32, tag="iota_i")
            nc.gpsimd.iota(iota_i[:], pattern=[[1, C]], base=0,
                           channel_multiplier=0)
            iota_c = pp.tile([P, C], f32, tag="iota_c")
            nc.vector.tensor_copy(iota_c[:], iota_i[:])

            ones_col = pp.tile([P, 1], f32, tag="ones_col")
            nc.vector.memset(ones_col[:], 1.0)
            ones_row = pp.tile([1, P], f32, tag="ones_row")
            nc.vector.memset(ones_row[:], 1.0)

            # ---------------- loads ----------------
            ch_sb = pp.tile([P, NB], i32, tag="ch_sb")
            nc.sync.dma_start(ch_sb[:], ch_d[:])
            ch_f = pp.tile([P, NB], f32, tag="ch_f")
            nc.vector.tensor_copy(ch_f[:], ch_sb[:])

            # bias as a single-partition row so matmul rhs has base partition 0
            bias_row = pp.tile([1, C * D], f32, tag="bias_row")
            nc.sync.dma_start(bias_row[:], b_d[:])

            x_sb = pp.tile([P, NB * D], f32, tag="x_sb")
            for j in range(NB):
                nc.sync.dma_start(x_sb[:, j * D:(j + 1) * D],
                                  x_d[j * P:(j + 1) * P, :])

            WT = pp.tile([D, C * D], f32, tag="WT")  # W^T: [i, (c,o)]

            O_all = pp.tile([P, NB * C], f32, tag="O_all")
            slotsel = pp.tile([P, NB], f32, tag="slotsel")

            with tc.tile_pool(name="psA", bufs=2, space="PSUM") as psA, \
                 tc.tile_pool(name="psB", bufs=1, space="PSUM") as psB:

                # W transposes: w rows (c,o) -> WT cols
                for r in range(NB):
                    w_t = wl.tile([P, D], f32, tag="w_t")
                    nc.sync.dma_start(w_t[:], w_d[r * P:(r + 1) * P, :])
                    wt_ps = psA.tile([D, P], f32, tag="wt")
                    nc.tensor.transpose(wt_ps[:], w_t[:], ident[:])
                    nc.vector.tensor_copy(WT[:, r * P:(r + 1) * P], wt_ps[:])

                # onehots per block
                for f in range(NB):
                    nc.vector.tensor_tensor(
                        out=O_all[:, f * C:(f + 1) * C],
                        in0=iota_c[:],
                        in1=ch_f[:, f:f + 1].to_broadcast([P, C]),
                        op=mybir.AluOpType.is_equal,
                    )

                # per-(block, channel) counts -> cntT [c, b]
                cntT_ps = psB.tile([C, NB], f32, tag="cntT")
                for f in range(NB):
                    nc.tensor.matmul(cntT_ps[:, f:f + 1],
                                     lhsT=O_all[:, f * C:(f + 1) * C],
                                     rhs=ones_col[:], start=True, stop=True)
                cntT_sb = tp.tile([C, NB], f32, tag="cntT_sb")
                nc.vector.tensor_copy(cntT_sb[:], cntT_ps[:])

                cnt_ps = psB.tile([NB, C], f32, tag="cnt")
                nc.tensor.transpose(cnt_ps[:], cntT_sb[:], ident[0:C, 0:C])
                cnt_sb = tp.tile([NB, C], f32, tag="cnt_sb")
                nc.vector.tensor_copy(cnt_sb[:], cnt_ps[:])

                # exclusive prefix over blocks
                boff_ps = psB.tile([NB, C], f32, tag="boff")
                nc.tensor.matmul(boff_ps[:], lhsT=u32[:], rhs=cnt_sb[:],
                                 start=True, stop=True)
                boff_sb = tp.tile([NB, C], f32, tag="boff_sb")
                nc.vector.tensor_copy(boff_sb[:], boff_ps[:])
                # flatten to a single-partition row for matmul rhs (base 0)
                boff_row = pp.tile([1, NB * C], f32, tag="boff_row")
                nc.sync.dma_start(boff_row[:], boff_sb[:])

                # global rank per token + select
                for f in range(NB):
                    r_ps = psA.tile([P, C], f32, tag="r")
                    nc.tensor.matmul(r_ps[:], lhsT=u128[:],
                                     rhs=O_all[:, f * C:(f + 1) * C],
                                     start=True, stop=False)
                    nc.tensor.matmul(r_ps[:], lhsT=ones_row[:],
                                     rhs=boff_row[0:1, f * C:(f + 1) * C],
                                     start=False, stop=True)
                    sel = tp.tile([P, C], f32, tag="sel")
                    nc.vector.tensor_tensor(out=sel[:], in0=r_ps[:],
                                            in1=O_all[:, f * C:(f + 1) * C],
                                            op=mybir.AluOpType.mult)
                    nc.vector.tensor_reduce(out=slotsel[:, f:f + 1],
                                            in_=sel[:],
                                            axis=mybir.AxisListType.X,
                                            op=mybir.AluOpType.add)

            # dest = channel*128 + slot
            ch128 = pp.tile([P, NB], f32, tag="ch128")
            nc.vector.tensor_scalar_mul(ch128[:], ch_f[:], float(P))
            dest_f = pp.tile([P, NB], f32, tag="dest_f")
            nc.vector.tensor_tensor(out=dest_f[:], in0=slotsel[:],
                                    in1=ch128[:], op=mybir.AluOpType.add)
            dest_i = pp.tile([P, NB], i32, tag="dest_i")
            nc.vector.tensor_copy(dest_i[:], dest_f[:])

            # ---------------- scatter x into sorted table ----------------
            for j in range(NB):
                nc.gpsimd.indirect_dma_start(
                    out=xs_d[:],
                    out_offset=bass.IndirectOffsetOnAxis(
                        ap=dest_i[:, j:j + 1], axis=0),
                    in_=x_sb[:, j * D:(j + 1) * D],
                    in_offset=None,
                )

            # ---------------- load sorted x ----------------
            xsort = pp.tile([P, C * D], f32, tag="xsort")
            for e in range(C):
                nc.sync.dma_start(xsort[:, e * D:(e + 1) * D],
                                  xs_d[e * P:(e + 1) * P, :])

            # ---------------- per-expert GEMM ----------------
            y_all = pp.tile([P, C * D], f32, tag="y_all")
            with tc.tile_pool(name="psC", bufs=2, space="PSUM") as psC, \
                 tc.tile_pool(name="psD", bufs=4, space="PSUM") as psD, \
                 tc.tile_pool(name="xTp", bufs=3) as xTp:
                for pr in range(C // 2):
                    tp_ps = psC.tile([P, P], f32, tag="tp")
                    nc.tensor.transpose(
                        tp_ps[:], xsort[:, pr * P:(pr + 1) * P], ident[:])
                    # split halves into base-0 tiles (matmul base-partition rule)
                    xT_a = xTp.tile([D, P], f32, tag="xT_a")
                    nc.vector.tensor_copy(xT_a[:], tp_ps[0:D, :])
                    xT_b = xTp.tile([D, P], f32, tag="xT_b")
                    nc.vector.tensor_copy(xT_b[:], tp_ps[D:P, :])
                    for h in range(2):
                        e = 2 * pr + h
                        xT = xT_a if h == 0 else xT_b
                        y_ps = psD.tile([P, D], f32, tag="y")
                        nc.tensor.matmul(y_ps[:], lhsT=xT[:],
                                         rhs=WT[:, e * D:(e + 1) * D],
                                         start=True, stop=False)
                        nc.tensor.matmul(y_ps[:], lhsT=ones_row[:],
                                         rhs=bias_row[0:1, e * D:(e + 1) * D],
                                         start=False, stop=True)
                        ysl = y_all[:, e * D:(e + 1) * D]
                        nc.scalar.activation(
                            out=ysl, in_=y_ps[:],
                            func=mybir.ActivationFunctionType.Tanh)
                        nc.vector.tensor_tensor(
                            out=ysl, in0=ysl,
                            in1=xsort[:, e * D:(e + 1) * D],
                            op=mybir.AluOpType.add)
                        nc.sync.dma_start(ys_d[e * P:(e + 1) * P, :], ysl)

            # ---------------- gather y back to token order ----------------
            y_sb = pp.tile([P, NB * D], f32, tag="y_sb")
            for j in range(NB):
                nc.gpsimd.indirect_dma_start(
                    out=y_sb[:, j * D:(j + 1) * D],
                    out_offset=None,
                    in_=ys_d[:],
                    in_offset=bass.IndirectOffsetOnAxis(
                        ap=dest_i[:, j:j + 1], axis=0),
                )
                nc.sync.dma_start(y_d[j * P:(j + 1) * P, :],
                                  y_sb[:, j * D:(j + 1) * D])

    nc.compile()
    return nc


def kernel(x, channels, weight, bias):
    global _COMPILED, LAST_RESULTS
    x = np.asarray(x)
    channels_in = np.asarray(channels)
    weight = np.asarray(weight)
    bias = np.asarray(bias)

    if _COMPILED is None:
        _COMPILED = _build()
    nc = _COMPILED

    B = x.shape[0]                      # 32
    bpc = B // NCORES                   # batches per core
    xf = np.ascontiguousarray(x.reshape(NCORES, T, D), dtype=np.float32)
    chf = channels_in.reshape(NCORES, T).astype(np.int32)
    w2 = np.ascontiguousarray(weight.reshape(C * D, D), dtype=np.float32)
    b2 = np.ascontiguousarray(bias, dtype=np.float32)

    in_maps = []
    for i in range(NCORES):
        # ch layout [p, f] with token t = f*128 + p
        ch2 = np.ascontiguousarray(chf[i].reshape(NB, P).T)
        in_maps.append({"x": xf[i], "ch": ch2, "w": w2, "b": b2})

    res = run_bass_kernel_spmd(nc, in_maps, list(range(NCORES)), trace=TRACE)
    LAST_RESULTS = res

    y = np.stack([res.results[i]["y"] for i in range(NCORES)])
    y = y.reshape(B, x.shape[1], D)
    return y, channels_in


# revision 9
# speedup vs baseline: 1.3241x; 1.3241x over previous
"""MoE routing kernel (nn_Dense_69045894250875) for 8 Trainium2 NeuronCores.

reference:  y = tanh(einsum('bloi,bli->blo', weight[channels], x) + bias[channels]) + x
            returns (y, channels)

Strategy (data-parallel over batch, 4 batches = 4096 tokens per core):
  1. On-device counting sort of tokens by channel: ranks via strict-upper
     triangular matmuls (bf16 inputs, fp32 psum - exact for 0/1 values) +
     per-block histogram prefix; select via onehot mult+reduce on VectorE.
  2. Indirect-DMA scatter of x rows into a capacity-padded sorted DRAM
     table (128 slots per expert), 32 calls of 128 rows.
  3. Per-expert GEMM on PE in fp32 (exact): lhsT = [x_e^T ; ones] (K=65),
     rhs = [W_e^T ; bias_e] so bias comes free; tanh on ScalarE; residual
     add on VectorE.
  4. Dense store of sorted y, indirect-DMA gather back into token order.

Host only reshapes/shards inputs (including passing weight pre-transposed
to [c, i, o] layout) and reassembles the output.
"""
import sys

sys.path.insert(0, "/opt/trn_rl_repo")

import numpy as np

import concourse.bass as bass
import concourse.mybir as mybir
import concourse.tile as tile
from concourse import bacc
from concourse.bass_utils import run_bass_kernel_spmd

P = 128          # partitions / slots per expert (capacity)
NB = 32          # token blocks per core
T = P * NB       # tokens per core = 4096
C = 64           # experts
D = 64           # in/out features
S = C * P        # sorted table slots = 8192
NCORES = 8

f32 = mybir.dt.float32
bf16 = mybir.dt.bfloat16
i32 = mybir.dt.int32

TRACE = False          # test harness sets True (requires NTFF hook installed)
LAST_RESULTS = None    # test harness reads this

_COMPILED = None


def _strict_upper(nc, ap, n):
    """ap[k, m] = 1 if k < m else 0  (k = partition, m = free)."""
    nc.vector.memset(ap, 0.0)
    nc.gpsimd.affine_select(
        out=ap, in_=ap,
        compare_op=mybir.AluOpType.is_ge,   # keep 0 where k-m >= 0, else fill 1
        fill=1.0, base=0,
        pattern=[[-1, n]], channel_multiplier=1,
    )


def _build():
    nc = bacc.Bacc("TRN2", target_bir_lowering=False, debug=False,
                   num_devices=NCORES)

    x_d = nc.dram_tensor("x", [T, D], f32, kind="ExternalInput")
    ch_d = nc.dram_tensor("ch", [P, NB], i32, kind="ExternalInput")
    w_d = nc.dram_tensor("w", [C * D, D], f32, kind="ExternalInput")  # [(c,i), o]
    b_d = nc.dram_tensor("b", [C, D], f32, kind="ExternalInput")
    y_d = nc.dram_tensor("y", [T, D], f32, kind="ExternalOutput")

    xs_d = nc.dram_tensor("xs_scratch", [S, D], f32)   # sorted x (padded)
    ys_d = nc.dram_tensor("ys_scratch", [S, D], f32)   # sorted y (padded)

    with tile.TileContext(nc) as tc:
        with tc.tile_pool(name="persist", bufs=1) as pp, \
             tc.tile_pool(name="tmp", bufs=3) as tp:

            # ---------------- constants (VectorE so GpSimd stays free) ------
            ident = pp.tile([P, P], f32, tag="ident")
            nc.vector.memset(ident[:], 0.0)
            nc.gpsimd.affine_select(
                out=ident[:], in_=ident[:],
                compare_op=mybir.AluOpType.not_equal, fill=1.0, base=0,
                pattern=[[-1, P]], channel_multiplier=1)

            u128 = pp.tile([P, P], bf16, tag="u128")
            _strict_upper(nc, u128[:], P)
            u32 = pp.tile([NB, NB], f32, tag="u32")
            _strict_upper(nc, u32[:], NB)

            iota_i = pp.tile([P, C], i32, tag="iota_i")
            nc.gpsimd.iota(iota_i[:], pattern=[[1, C]], base=0,
                           channel_multiplier=0)
            iota_c = pp.tile([P, C], f32, tag="iota_c")
            nc.vector.tensor_copy(iota_c[:], iota_i[:])

            ones_col = pp.tile([P, 1], bf16, tag="ones_col")
            nc.vector.memset(ones_col[:], 1.0)
            ones_row = pp.tile([1, P], f32, tag="ones_row")
            nc.vector.memset(ones_row[:], 1.0)

            # ---------------- loads ----------------
            ch_sb = pp.tile([P, NB], i32, tag="ch_sb")
            nc.sync.dma_start(ch_sb[:], ch_d[:])
            ch_f = pp.tile([P, NB], f32, tag="ch_f")
            nc.vector.tensor_copy(ch_f[:], ch_sb[:])

            # W^T + bias, K-augmented: rows 0..63 = W_e^T, row 64 = bias_e
            WT = pp.tile([D + 1, C * D], f32, tag="WT")
            nc.sync.dma_start(
                WT[0:D, :].rearrange("i (c o) -> i c o", c=C),
                w_d[:].rearrange("(c i) o -> i c o", i=D))
            nc.sync.dma_start(WT[D:D + 1, :], b_d[:])

            x_sb = pp.tile([P, NB * D], f32, tag="x_sb")
            nc.sync.dma_start(
                x_sb[:].rearrange("p (f o) -> p f o", f=NB),
                x_d[:].rearrange("(f p) o -> p f o", p=P))

            # ---------------- routing ----------------
            O_all = pp.tile([P, NB * C], bf16, tag="O_all")
            for f in range(NB):
                nc.vector.tensor_tensor(
                    out=O_all[:, f * C:(f + 1) * C],
                    in0=iota_c[:],
                    in1=ch_f[:, f:f + 1].to_broadcast([P, C]),
                    op=mybir.AluOpType.is_equal,
                )

            slotsel = pp.tile([P, NB], f32, tag="slotsel")

            with tc.tile_pool(name="psR", bufs=2, space="PSUM") as psR, \
                 tc.tile_pool(name="psB", bufs=1, space="PSUM") as psB:

                # per-(block, channel) counts -> cntT [c, b]
                cntT_ps = psB.tile([C, NB], f32, tag="cntT")
                for f in range(NB):
                    nc.tensor.matmul(cntT_ps[:, f:f + 1],
                                     lhsT=O_all[:, f * C:(f + 1) * C],
                                     rhs=ones_col[:], start=True, stop=True)
                cntT_sb = tp.tile([C, NB], f32, tag="cntT_sb")
                nc.vector.tensor_copy(cntT_sb[:], cntT_ps[:])

                cnt_ps = psB.tile([NB, C], f32, tag="cnt")
                nc.tensor.transpose(cnt_ps[:], cntT_sb[:], ident[0:C, 0:C])
                cnt_sb = tp.tile([NB, C], f32, tag="cnt_sb")
                nc.vector.tensor_copy(cnt_sb[:], cnt_ps[:])

                # exclusive prefix over blocks -> boff [b, c], then flat row
                boff_ps = psB.tile([NB, C], f32, tag="boff")
                nc.tensor.matmul(boff_ps[:], lhsT=u32[:], rhs=cnt_sb[:],
                                 start=True, stop=True)
                boff_sb = tp.tile([NB, C], f32, tag="boff_sb")
                nc.vector.tensor_copy(boff_sb[:], boff_ps[:])
                boff_row = pp.tile([1, NB * C], f32, tag="boff_row")
                nc.sync.dma_start(boff_row[:], boff_sb[:])

                # within-block ranks, 4 batched matmuls of N=512
                NCHUNK = 4
                W_CH = NB * C // NCHUNK          # 512 columns per chunk
                BL_CH = W_CH // C                # 8 blocks per chunk
                for q in range(NCHUNK):
                    r_ps = psR.tile([P, W_CH], f32, tag="r")
                    # rank within block (bf16 0/1 inputs, exact) ...
                    nc.tensor.matmul(
                        r_ps[:], lhsT=u128[:],
                        rhs=O_all[:, q * W_CH:(q + 1) * W_CH],
                        start=True, stop=False)
                    # ... + broadcast of block offsets (fp32)
                    nc.tensor.matmul(
                        r_ps[:], lhsT=ones_row[:],
                        rhs=boff_row[0:1, q * W_CH:(q + 1) * W_CH],
                        start=False, stop=True)
                    for fb in range(BL_CH):
                        f = q * BL_CH + fb
                        rsl = r_ps[:, fb * C:(fb + 1) * C]
                        osl = O_all[:, f * C:(f + 1) * C]
                        tmp = tp.tile([P, C], f32, tag="sel")
                        nc.vector.tensor_tensor(
                            out=tmp[:], in0=rsl, in1=osl,
                            op=mybir.AluOpType.mult)
                        nc.vector.tensor_reduce(
                            out=slotsel[:, f:f + 1], in_=tmp[:],
                            axis=mybir.AxisListType.X,
                            op=mybir.AluOpType.add)

            # dest = channel*128 + slot
            ch128 = pp.tile([P, NB], f32, tag="ch128")
            nc.vector.tensor_scalar_mul(ch128[:], ch_f[:], float(P))
            dest_f = pp.tile([P, NB], f32, tag="dest_f")
            nc.vector.tensor_tensor(out=dest_f[:], in0=slotsel[:],
                                    in1=ch128[:], op=mybir.AluOpType.add)
            dest_i = pp.tile([P, NB], i32, tag="dest_i")
            nc.vector.tensor_copy(dest_i[:], dest_f[:])

            # ---------------- scatter x into sorted table ----------------
            for j in range(NB):
                nc.gpsimd.indirect_dma_start(
                    out=xs_d[:],
                    out_offset=bass.IndirectOffsetOnAxis(
                        ap=dest_i[:, j:j + 1], axis=0),
                    in_=x_sb[:, j * D:(j + 1) * D],
                    in_offset=None,
                )

            # ---------------- load sorted x (slot-major) ----------------
            xsort = pp.tile([P, C * D], f32, tag="xsort")
            nc.sync.dma_start(
                xsort[:].rearrange("k (e o) -> k e o", e=C),
                xs_d[:].rearrange("(e k) o -> k e o", k=P))

            # ---------------- per-expert GEMM ----------------
            y_all = pp.tile([P, C * D], f32, tag="y_all")
            with tc.tile_pool(name="psC", bufs=2, space="PSUM") as psC, \
                 tc.tile_pool(name="psD", bufs=4, space="PSUM") as psD, \
                 tc.tile_pool(name="xTp", bufs=3) as xTp:
                for pr in range(C // 2):
                    tp_ps = psC.tile([P, P], f32, tag="tp")
                    nc.tensor.transpose(
                        tp_ps[:], xsort[:, pr * P:(pr + 1) * P], ident[:])
                    # [65, 256]: cols h*128.. = expert 2pr+h, row 64 = ones
                    xTt = xTp.tile([D + 1, 2 * P], f32, tag="xTt")
                    nc.vector.tensor_copy(xTt[0:D, 0:P], tp_ps[0:D, :])
                    nc.vector.tensor_copy(xTt[0:D, P:2 * P], tp_ps[D:P, :])
                    nc.vector.memset(xTt[D:D + 1, :], 1.0)
                    for h in range(2):
                        e = 2 * pr + h
                        y_ps = psD.tile([P, D], f32, tag="y")
                        nc.tensor.matmul(y_ps[:],
                                         lhsT=xTt[:, h * P:(h + 1) * P],
                                         rhs=WT[:, e * D:(e + 1) * D],
                                         start=True, stop=True)
                        ysl = y_all[:, e * D:(e + 1) * D]
                        nc.scalar.activation(
                            out=ysl, in_=y_ps[:],
                            func=mybir.ActivationFunctionType.Tanh)
                        nc.vector.tensor_tensor(
                            out=ysl, in0=ysl,
                            in1=xsort[:, e * D:(e + 1) * D],
                            op=mybir.AluOpType.add)
                    # store a pair of experts at a time
                    nc.sync.dma_start(
                        ys_d[pr * 2 * P:(pr + 1) * 2 * P, :]
                            .rearrange("(e k) o -> k e o", k=P),
                        y_all[:, pr * P:(pr + 1) * P]
                            .rearrange("k (e o) -> k e o", e=2))

            # ---------------- gather y back to token order ----------------
            y_sb = pp.tile([P, NB * D], f32, tag="y_sb")
            for j in range(NB):
                nc.gpsimd.indirect_dma_start(
                    out=y_sb[:, j * D:(j + 1) * D],
                    out_offset=None,
                    in_=ys_d[:],
                    in_offset=bass.IndirectOffsetOnAxis(
                        ap=dest_i[:, j:j + 1], axis=0),
                )
                if j % 8 == 7:
                    g = j // 8
                    nc.sync.dma_start(
                        y_d[g * 8 * P:(g + 1) * 8 * P, :]
                            .rearrange("(f p) o -> p f o", p=P),
                        y_sb[:, g * 8 * D:(g + 1) * 8 * D]
                            .rearrange("p (f o) -> p f o", f=8))

    nc.compile()
    return nc


def kernel(x, channels, weight, bias):
    global _COMPILED, LAST_RESULTS
    x = np.asarray(x)
    channels_in = np.asarray(channels)
    weight = np.asarray(weight)
    bias = np.asarray(bias)

    if _COMPILED is None:
        _COMPILED = _build()
    nc = _COMPILED

    B = x.shape[0]                      # 32
    xf = np.ascontiguousarray(x.reshape(NCORES, T, D), dtype=np.float32)
    chf = channels_in.reshape(NCORES, T).astype(np.int32)
    # pre-transposed weights: [(c, i), o]
    w2 = np.ascontiguousarray(
        weight.transpose(0, 2, 1).reshape(C * D, D).astype(np.float32))
    b2 = np.ascontiguousarray(bias, dtype=np.float32)

    in_maps = []
    for i in range(NCORES):
        # ch layout [p, f] with token t = f*128 + p
        ch2 = np.ascontiguousarray(chf[i].reshape(NB, P).T)
        in_maps.append({"x": xf[i], "ch": ch2, "w": w2, "b": b2})

    res = run_bass_kernel_spmd(nc, in_maps, list(range(NCORES)), trace=TRACE)
    LAST_RESULTS = res

    y = np.stack([res.results[i]["y"] for i in range(NCORES)])
    y = y.reshape(B, x.shape[1], D)
    return y, channels_in


# revision 11
# speedup vs baseline: 1.6661x; 1.2583x over previous
"""MoE routing kernel (nn_Dense_69045894250875) for 8 Trainium2 NeuronCores.

reference:  y = tanh(einsum('bloi,bli->blo', weight[channels], x) + bias[channels]) + x
            returns (y, channels)

Strategy (data-parallel over batch, 4 batches = 4096 tokens per core):
  1. On-device counting sort of tokens by channel: ranks via strict-upper
     triangular matmuls (bf16 0/1 inputs, fp32 psum - exact) + per-block
     histogram prefix; slot select via batched onehot mult+reduce on DVE.
  2. Indirect-DMA scatter of x rows into a capacity-padded sorted DRAM
     table (128 slots/expert), 32 calls of 128 rows issued back-to-back
     inside a tile_critical (same-queue FIFO makes this safe), completion
     via one shared semaphore.
  3. Per-expert GEMM on PE in fp32 (exact): lhsT = [x_e^T ; ones] (K=65),
     rhs = [W_e^T ; bias_e] so bias is free; tanh pairs on ScalarE;
     residual add pairs on GpSimd.
  4. Dense store of sorted y, indirect-DMA gather back into token order.

Host only reshapes/shards inputs (weight passed pre-transposed [c, i, o])
and reassembles the output.
"""
import sys

sys.path.insert(0, "/opt/trn_rl_repo")

import numpy as np

import concourse.bass as bass
import concourse.mybir as mybir
import concourse.tile as tile
from concourse import bacc
from concourse.bass_utils import run_bass_kernel_spmd

P = 128          # partitions / slots per expert (capacity)
NB = 32          # token blocks per core
T = P * NB       # tokens per core = 4096
C = 64           # experts
D = 64           # in/out features
S = C * P        # sorted table slots = 8192
NCORES = 8

f32 = mybir.dt.float32
bf16 = mybir.dt.bfloat16
i32 = mybir.dt.int32

TRACE = False          # test harness sets True (requires NTFF hook installed)
LAST_RESULTS = None    # test harness reads this

_COMPILED = None


def _strict_upper(nc, ap, n):
    """ap[k, m] = 1 if k < m else 0  (k = partition, m = free)."""
    nc.vector.memset(ap, 0.0)
    nc.gpsimd.affine_select(
        out=ap, in_=ap,
        compare_op=mybir.AluOpType.is_ge,   # keep 0 where k-m >= 0, else fill 1
        fill=1.0, base=0,
        pattern=[[-1, n]], channel_multiplier=1,
    )


def _build():
    nc = bacc.Bacc("TRN2", target_bir_lowering=False, debug=False,
                   num_devices=NCORES)

    x_d = nc.dram_tensor("x", [T, D], f32, kind="ExternalInput")
    ch_d = nc.dram_tensor("ch", [P, NB], i32, kind="ExternalInput")
    w_d = nc.dram_tensor("w", [C * D, D], f32, kind="ExternalInput")  # [(c,i), o]
    b_d = nc.dram_tensor("b", [C, D], f32, kind="ExternalInput")
    y_d = nc.dram_tensor("y", [T, D], f32, kind="ExternalOutput")

    xs_d = nc.dram_tensor("xs_scratch", [S, D], f32)   # sorted x (padded)
    ys_d = nc.dram_tensor("ys_scratch", [S, D], f32)   # sorted y (padded)

    with tile.TileContext(nc) as tc:
        with tc.tile_pool(name="persist", bufs=1) as pp, \
             tc.tile_pool(name="tmp", bufs=3) as tp:

            # ---------------- constants ----------------
            ident = pp.tile([P, P], f32, tag="ident")
            nc.vector.memset(ident[:], 0.0)
            nc.gpsimd.affine_select(
                out=ident[:], in_=ident[:],
                compare_op=mybir.AluOpType.not_equal, fill=1.0, base=0,
                pattern=[[-1, P]], channel_multiplier=1)

            u128 = pp.tile([P, P], bf16, tag="u128")
            _strict_upper(nc, u128[:], P)
            u32 = pp.tile([NB, NB], f32, tag="u32")
            _strict_upper(nc, u32[:], NB)

            # iota over channels, repeated per block: [p, (f, c)] = c
            iota_fc = pp.tile([P, NB * C], f32, tag="iota_fc")
            nc.gpsimd.iota(iota_fc[:], pattern=[[0, NB], [1, C]], base=0,
                           channel_multiplier=0,
                           allow_small_or_imprecise_dtypes=True)

            ones_col = pp.tile([P, 1], bf16, tag="ones_col")
            nc.vector.memset(ones_col[:], 1.0)
            ones2 = pp.tile([2, P], bf16, tag="ones2")
            nc.vector.memset(ones2[:], 1.0)

            # persistent transposed-x with a built-in ones row (K=65 aug)
            xTt = pp.tile([D + 1, C * P], f32, tag="xTt")
            nc.gpsimd.memset(xTt[D:D + 1, :], 1.0)

            # ---------------- loads ----------------
            ch_sb = pp.tile([P, NB], i32, tag="ch_sb")
            nc.sync.dma_start(ch_sb[:], ch_d[:])
            ch_f = pp.tile([P, NB], f32, tag="ch_f")
            nc.vector.tensor_copy(ch_f[:], ch_sb[:])

            # W^T + bias, K-augmented: rows 0..63 = W_e^T, row 64 = bias_e
            WT = pp.tile([D + 1, C * D], f32, tag="WT")
            nc.sync.dma_start(
                WT[0:D, :].rearrange("i (c o) -> i c o", c=C),
                w_d[:].rearrange("(c i) o -> i c o", i=D))
            nc.sync.dma_start(WT[D:D + 1, :], b_d[:])

            x_sb = pp.tile([P, NB * D], f32, tag="x_sb")
            nc.sync.dma_start(
                x_sb[:].rearrange("p (f o) -> p f o", f=NB),
                x_d[:].rearrange("(f p) o -> p f o", p=P))

            # ---------------- routing ----------------
            # one batched onehot build: O[p, (f, c)] = (c == ch[p, f])
            O_all = pp.tile([P, NB * C], bf16, tag="O_all")
            nc.vector.tensor_tensor(
                out=O_all[:].rearrange("p (f c) -> p f c", f=NB),
                in0=iota_fc[:].rearrange("p (f c) -> p f c", f=NB),
                in1=ch_f[:].rearrange("p (f c) -> p f c", c=1)
                    .to_broadcast([P, NB, C]),
                op=mybir.AluOpType.is_equal,
            )

            slotsel = pp.tile([P, NB], f32, tag="slotsel")

            with tc.tile_pool(name="psR", bufs=2, space="PSUM") as psR, \
                 tc.tile_pool(name="psB", bufs=1, space="PSUM") as psB:

                # per-(block, channel) counts -> cntT [c, b]
                cntT_ps = psB.tile([C, NB], f32, tag="cntT")
                for f in range(NB):
                    nc.tensor.matmul(cntT_ps[:, f:f + 1],
                                     lhsT=O_all[:, f * C:(f + 1) * C],
                                     rhs=ones_col[:], start=True, stop=True)
                cntT_sb = tp.tile([C, NB], f32, tag="cntT_sb")
                nc.vector.tensor_copy(cntT_sb[:], cntT_ps[:])

                cnt_ps = psB.tile([NB, C], f32, tag="cnt")
                nc.tensor.transpose(cnt_ps[:], cntT_sb[:], ident[0:C, 0:C])
                cnt_sb = tp.tile([NB, C], f32, tag="cnt_sb")
                nc.vector.tensor_copy(cnt_sb[:], cnt_ps[:])

                # exclusive prefix over blocks -> boff [b, c]
                boff_ps = psB.tile([NB, C], f32, tag="boff")
                nc.tensor.matmul(boff_ps[:], lhsT=u32[:], rhs=cnt_sb[:],
                                 start=True, stop=True)
                # split hi/lo so the broadcast matmul can run in bf16 exactly
                boff_i = tp.tile([NB, C], i32, tag="boff_i")
                nc.vector.tensor_copy(boff_i[:], boff_ps[:])
                lo_i = tp.tile([NB, C], i32, tag="lo_i")
                nc.vector.tensor_scalar(out=lo_i[:], in0=boff_i[:],
                                        scalar1=P - 1, scalar2=None,
                                        op0=mybir.AluOpType.bitwise_and)
                hi_i = tp.tile([NB, C], i32, tag="hi_i")
                nc.vector.tensor_tensor(out=hi_i[:], in0=boff_i[:],
                                        in1=lo_i[:],
                                        op=mybir.AluOpType.subtract)
                boff_lo = tp.tile([NB, C], bf16, tag="boff_lo")
                nc.vector.tensor_copy(boff_lo[:], lo_i[:])
                boff_hi = tp.tile([NB, C], bf16, tag="boff_hi")
                nc.vector.tensor_copy(boff_hi[:], hi_i[:])
                boff_rows = pp.tile([2, NB * C], bf16, tag="boff_rows")
                nc.sync.dma_start(boff_rows[0:1, :], boff_hi[:])
                nc.sync.dma_start(boff_rows[1:2, :], boff_lo[:])

                # global slot per token: rank-within-block + block offset,
                # then pick own channel's column via onehot mult+reduce
                NCHUNK = 4
                W_CH = NB * C // NCHUNK          # 512 columns per chunk
                BL_CH = W_CH // C                # 8 blocks per chunk
                for q in range(NCHUNK):
                    r_ps = psR.tile([P, W_CH], f32, tag="r")
                    nc.tensor.matmul(
                        r_ps[:], lhsT=u128[:],
                        rhs=O_all[:, q * W_CH:(q + 1) * W_CH],
                        start=True, stop=False)
                    nc.tensor.matmul(
                        r_ps[:], lhsT=ones2[:],
                        rhs=boff_rows[:, q * W_CH:(q + 1) * W_CH],
                        start=False, stop=True)
                    sel = tp.tile([P, W_CH], f32, tag="sel")
                    nc.vector.tensor_tensor(
                        out=sel[:], in0=r_ps[:],
                        in1=O_all[:, q * W_CH:(q + 1) * W_CH],
                        op=mybir.AluOpType.mult)
                    nc.vector.tensor_reduce(
                        out=slotsel[:, q * BL_CH:(q + 1) * BL_CH],
                        in_=sel[:].rearrange("p (f c) -> p f c", c=C),
                        axis=mybir.AxisListType.X,
                        op=mybir.AluOpType.add)

            # dest = channel*128 + slot
            ch128 = pp.tile([P, NB], f32, tag="ch128")
            nc.vector.tensor_scalar_mul(ch128[:], ch_f[:], float(P))
            dest_f = pp.tile([P, NB], f32, tag="dest_f")
            nc.vector.tensor_tensor(out=dest_f[:], in0=slotsel[:],
                                    in1=ch128[:], op=mybir.AluOpType.add)
            dest_i = pp.tile([P, NB], i32, tag="dest_i")
            nc.vector.tensor_copy(dest_i[:], dest_f[:])

            # ---------------- scatter x into sorted table ----------------
            # back-to-back issue inside a critical section; same SWDGE queue
            # drains FIFO per engine, so waiting on the shared semaphore at
            # the end guarantees all rows have landed.
            scat_sem = nc.alloc_semaphore("scat_sem")
            with tc.tile_critical():
                for j in range(NB):
                    nc.gpsimd.indirect_dma_start(
                        out=xs_d[:],
                        out_offset=bass.IndirectOffsetOnAxis(
                            ap=dest_i[:, j:j + 1], axis=0),
                        in_=x_sb[:, j * D:(j + 1) * D],
                        in_offset=None,
                    ).then_inc(scat_sem, 16)
                nc.gpsimd.wait_ge(scat_sem, NB * 16)

            # ---------------- load sorted x (slot-major) ----------------
            xsort = pp.tile([P, C * D], f32, tag="xsort")
            nc.sync.dma_start(
                xsort[:].rearrange("k (e o) -> k e o", e=C),
                xs_d[:].rearrange("(e k) o -> k e o", k=P))

            # ---------------- per-expert GEMM ----------------
            y_all = pp.tile([P, C * D], f32, tag="y_all")
            with tc.tile_pool(name="psC", bufs=3, space="PSUM") as psC, \
                 tc.tile_pool(name="psD", bufs=3, space="PSUM") as psD:
                for pr in range(C // 2):
                    tp_ps = psC.tile([P, P], f32, tag="tp")
                    nc.tensor.transpose(
                        tp_ps[:], xsort[:, pr * P:(pr + 1) * P], ident[:])
                    nc.vector.tensor_copy(
                        xTt[0:D, pr * 2 * P:pr * 2 * P + P], tp_ps[0:D, :])
                    nc.vector.tensor_copy(
                        xTt[0:D, pr * 2 * P + P:(pr + 1) * 2 * P], tp_ps[D:P, :])
                    y_ps = psD.tile([P, P], f32, tag="y")
                    for h in range(2):
                        e = 2 * pr + h
                        nc.tensor.matmul(y_ps[:, h * D:(h + 1) * D],
                                         lhsT=xTt[:, e * P:(e + 1) * P],
                                         rhs=WT[:, e * D:(e + 1) * D],
                                         start=True, stop=True)
                    pr_sl = slice(pr * P, (pr + 1) * P)
                    nc.scalar.activation(
                        out=y_all[:, pr_sl], in_=y_ps[:],
                        func=mybir.ActivationFunctionType.Tanh)
                    nc.gpsimd.tensor_tensor(
                        out=y_all[:, pr_sl], in0=y_all[:, pr_sl],
                        in1=xsort[:, pr_sl], op=mybir.AluOpType.add)
                    nc.sync.dma_start(
                        ys_d[pr * 2 * P:(pr + 1) * 2 * P, :]
                            .rearrange("(e k) o -> k e o", k=P),
                        y_all[:, pr_sl].rearrange("k (e o) -> k e o", e=2))

            # ---------------- gather y back to token order ----------------
            y_sb = pp.tile([P, NB * D], f32, tag="y_sb")
            gath_sem = nc.alloc_semaphore("gath_sem")
            with tc.tile_critical():
                for j in range(NB):
                    nc.gpsimd.indirect_dma_start(
                        out=y_sb[:, j * D:(j + 1) * D],
                        out_offset=None,
                        in_=ys_d[:],
                        in_offset=bass.IndirectOffsetOnAxis(
                            ap=dest_i[:, j:j + 1], axis=0),
                    ).then_inc(gath_sem, 16)
                nc.gpsimd.wait_ge(gath_sem, NB * 16)

            for g in range(4):
                nc.sync.dma_start(
                    y_d[g * 8 * P:(g + 1) * 8 * P, :]
                        .rearrange("(f p) o -> p f o", p=P),
                    y_sb[:, g * 8 * D:(g + 1) * 8 * D]
                        .rearrange("p (f o) -> p f o", f=8))

    nc.compile()
    return nc


def kernel(x, channels, weight, bias):
    global _COMPILED, LAST_RESULTS
    x = np.asarray(x)
    channels_in = np.asarray(channels)
    weight = np.asarray(weight)
    bias = np.asarray(bias)

    if _COMPILED is None:
        _COMPILED = _build()
    nc = _COMPILED

    B = x.shape[0]                      # 32
    xf = np.ascontiguousarray(x.reshape(NCORES, T, D), dtype=np.float32)
    chf = channels_in.reshape(NCORES, T).astype(np.int32)
    # pre-transposed weights: [(c, i), o]
    w2 = np.ascontiguousarray(
        weight.transpose(0, 2, 1).reshape(C * D, D).astype(np.float32))
    b2 = np.ascontiguousarray(bias, dtype=np.float32)

    in_maps = []
    for i in range(NCORES):
        # ch layout [p, f] with token t = f*128 + p
        ch2 = np.ascontiguousarray(chf[i].reshape(NB, P).T)
        in_maps.append({"x": xf[i], "ch": ch2, "w": w2, "b": b2})

    res = run_bass_kernel_spmd(nc, in_maps, list(range(NCORES)), trace=TRACE)
    LAST_RESULTS = res

    y = np.stack([res.results[i]["y"] for i in range(NCORES)])
    y = y.reshape(B, x.shape[1], D)
    return y, channels_in
